# revision 26
# baseline (speedup 1.0000x reference)
"""Trainium2 Bass kernel for the Involution module (B=4, C=64, H=W=128, K=7, G=4).

v2 pixel-major architecture (8-way data parallel: core = (batch, h-half)):
  - partitions = 128 w-columns; free dim = (channel, row).
  - 1x1 kernel-generating conv runs TRANSPOSED on TensorE: lhsT = x-slice
    [65, 128] (64 channels + ones row), rhs = [65, 196] BN-folded weights;
    out z[128 px, 196 ko] in PSUM. SiLU on ScalarE -> bf16 `wall` laid out
    [p, (ko, r)] with r innermost (strided activation writes).
  - involution MAC on DVE: per k=(dh,dw), tensor_mul with the w operand read
    via a stride-0 broadcast AP over the 16 group-channels (no replication
    matmuls, no PSUM drain). dh shifts = free-dim offsets (odd dh uses an
    element-shifted DMA variant to keep bf16 2x alignment); dw shifts =
    DMA'd row-offset slabs from a 134-row padded DRAM image.
"""

import numpy as np
import ml_dtypes

import concourse.bacc as bacc
import concourse.tile as tile
import concourse.mybir as mybir
from concourse.bass_utils import run_bass_kernel_spmd

# Problem constants (hardcoded per harness contract).
B, C, H, W = 4, 64, 128, 128
K, G, GC = 7, 4, 16
KK = K * K
KO = KK * G  # 196
PAD = 3
BN_EPS = 1e-5

RPC = 64          # output rows per core
XR = RPC + 2 * PAD  # 70 rows incl. dh halo
XF = C * XR       # 4480 free elems per x slab partition
WCOL = W + 2 * PAD  # 134 padded w-columns in DRAM


def build_bass():
    nc = bacc.Bacc(
        "TRN2",
        target_bir_lowering=False,
        debug=False,
        enable_asserts=False,
        num_devices=8,
    )
    DT = mybir.dt.bfloat16
    f32 = mybir.dt.float32

    xpad_d = nc.dram_tensor("xpad", [WCOL, XF], DT, kind="ExternalInput").ap()
    xpod_d = nc.dram_tensor("xpod", [WCOL, XF], DT, kind="ExternalInput").ap()
    xcm_d = nc.dram_tensor("xcm", [C + 1, RPC * W], DT, kind="ExternalInput").ap()
    wconv_d = nc.dram_tensor("wconv", [C + 1, KO], DT, kind="ExternalInput").ap()
    out_d = nc.dram_tensor("out", [128, C * RPC], DT, kind="ExternalOutput").ap()

    with tile.TileContext(nc) as tc:
        build_kernel(tc, xpad_d, xpod_d, xcm_d, wconv_d, out_d)
    nc.compile()
    return nc


def build_kernel(tc, xpad_d, xpod_d, xcm_d, wconv_d, out_d):
    from contextlib import ExitStack

    nc = tc.nc
    DT = mybir.dt.bfloat16
    f32 = mybir.dt.float32
    silu = mybir.ActivationFunctionType.Silu

    ctx = ExitStack()
    consts = ctx.enter_context(tc.tile_pool(name="consts", bufs=1))
    slabs = ctx.enter_context(tc.tile_pool(name="slabs", bufs=4))
    tmppool = ctx.enter_context(tc.tile_pool(name="tmp", bufs=3))
    zpool = ctx.enter_context(tc.tile_pool(name="z", bufs=8, space="PSUM"))

    wconv = consts.tile([C + 1, KO], DT)
    nc.sync.dma_start(out=wconv, in_=wconv_d)
    # xcm as 16-row quarter tiles: conv rows start as soon as their slice lands
    NQ = 4
    QR = RPC // NQ
    xcmq = []
    for q in range(NQ):
        xq = consts.tile([C + 1, QR * W], DT, name=f"xcmq{q}")
        nc.sync.dma_start(out=xq, in_=xcm_d[:, q * QR * W : (q + 1) * QR * W])
        xcmq.append(xq)

    # conv: 64 transposed matmuls + SiLU into zbuf[p, (r, ko)] (contiguous act
    # writes), then DVE relayout to wall[p, (ko, r)] (r innermost for the
    # 2x-mode MAC), chunked so the transpose overlaps the remaining acts.
    zbuf = consts.tile([128, RPC * KO], DT)
    zbuf3 = zbuf.rearrange("p (r ko) -> p r ko", r=RPC)
    wall = consts.tile([128, KO * RPC], DT)
    wall3 = wall.rearrange("p (ko r) -> p ko r", r=RPC)
    RCH = 8
    for r0 in range(0, RPC, RCH):
        for r in range(r0, r0 + RCH):
            zr = zpool.tile([128, KO], f32, tag="z")
            nc.tensor.matmul(
                zr,
                xcmq[r // QR][:, (r % QR) * W : (r % QR + 1) * W],
                wconv,
                start=True,
                stop=True,
            )
            nc.scalar.activation(zbuf3[:, r : r + 1, :], zr, silu)
        # [RCH, KO] -> [KO, RCH] free-dim transpose (DVE 1x strided copy)
        nc.vector.tensor_copy(
            wall3[:, :, r0 : r0 + RCH],
            zbuf3[:, r0 : r0 + RCH, :].transpose([0, 2, 1]),
        )

    wall4 = wall.rearrange("p (g k r) -> p g k r", g=G, k=KK)

    acc0 = consts.tile([128, C * RPC], DT)
    acc1 = consts.tile([128, C * RPC], DT)

    # MAC: loop dw outer (DMA'd slab pair), dh inner
    first = {0: True, 1: True}
    for dw in range(K):
        xe = slabs.tile([128, XF], DT, tag="xe")
        nc.sync.dma_start(out=xe, in_=xpad_d[dw : dw + 128, :])
        xo = slabs.tile([128, XF], DT, tag="xo")
        nc.sync.dma_start(out=xo, in_=xpod_d[dw : dw + 128, :])
        xe3 = xe.rearrange("p (c r) -> p c r", r=XR)
        xo3 = xo.rearrange("p (c r) -> p c r", r=XR)
        for dh in range(K):
            k = dh * K + dw
            if dh % 2 == 0:
                xop = xe3[:, :, dh : dh + RPC]
            else:
                xop = xo3[:, :, dh - 1 : dh - 1 + RPC]
            wop = wall4[:, :, k : k + 1, :].broadcast_to([128, G, GC, RPC])
            a = acc0 if k % 2 == 0 else acc1
            if first[k % 2]:
                nc.vector.tensor_mul(a, xop, wop)
                first[k % 2] = False
            else:
                t = tmppool.tile([128, C * RPC], DT, tag="t")
                nc.vector.tensor_mul(t, xop, wop)
                nc.vector.tensor_add(a, a, t)

    outf = consts.tile([128, C * RPC], DT)
    HF = C * RPC // 2
    nc.vector.tensor_add(outf[:, 0:HF], acc0[:, 0:HF], acc1[:, 0:HF])
    nc.sync.dma_start(out=out_d[:, 0:HF], in_=outf[:, 0:HF])
    nc.vector.tensor_add(outf[:, HF:], acc0[:, HF:], acc1[:, HF:])
    nc.sync.dma_start(out=out_d[:, HF:], in_=outf[:, HF:])
    ctx.close()


def prep_inputs(x, conv_w, bn_gamma, bn_beta, bn_mean, bn_var):
    """Host-side prep: per-core padded pixel-major slabs + BN-folded weights."""
    bf = ml_dtypes.bfloat16
    scale = (bn_gamma / np.sqrt(bn_var + BN_EPS)).astype(np.float32)
    shift = (bn_beta - bn_mean * scale).astype(np.float32)

    # BN-folded transposed conv weights [65, 196]: rows 0..63 = (scale*W)^T,
    # row 64 = shift (pairs with the ones-row in xcm).
    wconv = np.zeros((C + 1, KO), np.float32)
    wconv[0:C] = (conv_w * scale[:, None]).T
    wconv[C] = shift
    wconv = wconv.astype(bf)

    # padded image [B, C, H+6, W+6]
    xp = np.zeros((B, C, H + 2 * PAD, W + 2 * PAD), np.float32)
    xp[:, :, PAD : PAD + H, PAD : PAD + W] = x

    in_maps = []
    for core in range(8):
        b, half = core // 2, core % 2
        h0 = 64 * half
        # xpad [134 wcols, (c, 70 rows)]: rows h0-3 .. h0+66 (padded idx h0..h0+69)
        slab = xp[b, :, h0 : h0 + XR, :]              # [C, 70, 134]
        xpad = np.ascontiguousarray(slab.transpose(2, 0, 1)).reshape(WCOL, XF)
        xpad = xpad.astype(bf)
        xpod = np.zeros_like(xpad)
        xpod[:, : XF - 1] = xpad[:, 1:]
        # xcm [65, (r, w)]: channels of the 64 output rows + ones row
        xcm = np.zeros((C + 1, RPC, W), np.float32)
        xcm[0:C] = x[b, :, h0 : h0 + RPC, :]
        xcm[C] = 1.0
        in_maps.append(
            {
                "xpad": xpad,
                "xpod": xpod,
                "xcm": xcm.reshape(C + 1, RPC * W).astype(bf),
                "wconv": wconv,
            }
        )
    return in_maps


def assemble_output(results):
    out = np.zeros((B, C, H, W), np.float32)
    for core in range(8):
        b, half = core // 2, core % 2
        h0 = 64 * half
        arr = results[core]["out"].astype(np.float32).reshape(128, C, RPC)  # [w, c, r]
        out[b, :, h0 : h0 + RPC, :] = arr.transpose(1, 2, 0)
    return out


def kernel(x, conv_w, bn_gamma, bn_beta, bn_mean, bn_var):
    x = np.asarray(x, np.float32)
    conv_w = np.asarray(conv_w, np.float32)
    in_maps = prep_inputs(
        x,
        conv_w,
        np.asarray(bn_gamma, np.float32),
        np.asarray(bn_beta, np.float32),
        np.asarray(bn_mean, np.float32),
        np.asarray(bn_var, np.float32),
    )
    nc = build_bass()
    res = run_bass_kernel_spmd(nc, in_maps, core_ids=list(range(8)))
    return assemble_output(res.results)


if __name__ == "__main__":
    rng = np.random.default_rng(0)
    ins = {
        "x": rng.standard_normal((B, C, H, W), np.float32),
        "conv_w": rng.standard_normal((KO, C), np.float32) / 8.0,
        "bn_gamma": rng.uniform(0.5, 1.5, KO).astype(np.float32),
        "bn_beta": rng.standard_normal(KO).astype(np.float32) * 0.1,
        "bn_mean": rng.standard_normal(KO).astype(np.float32) * 0.1,
        "bn_var": rng.uniform(0.5, 1.5, KO).astype(np.float32),
    }
    out = kernel(**ins)
    print("kernel output", out.shape, out.dtype, np.abs(out).sum())


# revision 27
# speedup vs baseline: 1.0011x; 1.0011x over previous
"""Trainium2 Bass kernel for the Involution module (B=4, C=64, H=W=128, K=7, G=4).

v2 pixel-major architecture (8-way data parallel: core = (batch, h-half)):
  - partitions = 128 w-columns; free dim = (channel, row).
  - 1x1 kernel-generating conv runs TRANSPOSED on TensorE: lhsT = x-slice
    [65, 128] (64 channels + ones row), rhs = [65, 196] BN-folded weights;
    out z[128 px, 196 ko] in PSUM. SiLU on ScalarE -> bf16 `wall` laid out
    [p, (ko, r)] with r innermost (strided activation writes).
  - involution MAC on DVE: per k=(dh,dw), tensor_mul with the w operand read
    via a stride-0 broadcast AP over the 16 group-channels (no replication
    matmuls, no PSUM drain). dh shifts = free-dim offsets (odd dh uses an
    element-shifted DMA variant to keep bf16 2x alignment); dw shifts =
    DMA'd row-offset slabs from a 134-row padded DRAM image.
"""

import numpy as np
import ml_dtypes

import concourse.bacc as bacc
import concourse.tile as tile
import concourse.mybir as mybir
from concourse.bass_utils import run_bass_kernel_spmd

# Problem constants (hardcoded per harness contract).
B, C, H, W = 4, 64, 128, 128
K, G, GC = 7, 4, 16
KK = K * K
KO = KK * G  # 196
PAD = 3
BN_EPS = 1e-5

RPC = 64          # output rows per core
XR = RPC + 2 * PAD  # 70 rows incl. dh halo
XF = C * XR       # 4480 free elems per x slab partition
WCOL = W + 2 * PAD  # 134 padded w-columns in DRAM


def build_bass():
    nc = bacc.Bacc(
        "TRN2",
        target_bir_lowering=False,
        debug=False,
        enable_asserts=False,
        num_devices=8,
    )
    DT = mybir.dt.bfloat16
    f32 = mybir.dt.float32

    xpad_d = nc.dram_tensor("xpad", [WCOL, XF], DT, kind="ExternalInput").ap()
    xpod_d = nc.dram_tensor("xpod", [WCOL, XF], DT, kind="ExternalInput").ap()
    xcm_d = nc.dram_tensor("xcm", [C + 1, RPC * W], DT, kind="ExternalInput").ap()
    wconv_d = nc.dram_tensor("wconv", [C + 1, KO], DT, kind="ExternalInput").ap()
    out_d = nc.dram_tensor("out", [128, C * RPC], DT, kind="ExternalOutput").ap()

    with tile.TileContext(nc) as tc:
        build_kernel(tc, xpad_d, xpod_d, xcm_d, wconv_d, out_d)
    nc.compile()
    return nc


def build_kernel(tc, xpad_d, xpod_d, xcm_d, wconv_d, out_d):
    from contextlib import ExitStack

    nc = tc.nc
    DT = mybir.dt.bfloat16
    f32 = mybir.dt.float32
    silu = mybir.ActivationFunctionType.Silu

    ctx = ExitStack()
    consts = ctx.enter_context(tc.tile_pool(name="consts", bufs=1))
    slabs = ctx.enter_context(tc.tile_pool(name="slabs", bufs=4))
    tmppool = ctx.enter_context(tc.tile_pool(name="tmp", bufs=3))
    zpool = ctx.enter_context(tc.tile_pool(name="z", bufs=8, space="PSUM"))

    wconv = consts.tile([C + 1, KO], DT)
    nc.sync.dma_start(out=wconv, in_=wconv_d)
    # xcm as 16-row quarter tiles: conv rows start as soon as their slice lands
    NQ = 4
    QR = RPC // NQ
    xcmq = []
    for q in range(NQ):
        xq = consts.tile([C + 1, QR * W], DT, name=f"xcmq{q}")
        nc.sync.dma_start(out=xq, in_=xcm_d[:, q * QR * W : (q + 1) * QR * W])
        xcmq.append(xq)

    # conv: 64 transposed matmuls + SiLU into zbuf[p, (r, ko)] (contiguous act
    # writes), then DVE relayout to wall[p, (ko, r)] (r innermost for the
    # 2x-mode MAC), chunked so the transpose overlaps the remaining acts.
    zbuf = consts.tile([128, RPC * KO], DT)
    zbuf3 = zbuf.rearrange("p (r ko) -> p r ko", r=RPC)
    wall = consts.tile([128, KO * RPC], DT)
    wall3 = wall.rearrange("p (ko r) -> p ko r", r=RPC)
    RCH = 8
    for r0 in range(0, RPC, RCH):
        for r in range(r0, r0 + RCH):
            zr = zpool.tile([128, KO], f32, tag="z")
            nc.tensor.matmul(
                zr,
                xcmq[r // QR][:, (r % QR) * W : (r % QR + 1) * W],
                wconv,
                start=True,
                stop=True,
            )
            nc.scalar.activation(zbuf3[:, r : r + 1, :], zr, silu)
        # [RCH, KO] -> [KO, RCH] free-dim transpose (DVE 1x strided copy)
        nc.vector.tensor_copy(
            wall3[:, :, r0 : r0 + RCH],
            zbuf3[:, r0 : r0 + RCH, :].transpose([0, 2, 1]),
        )

    wall4 = wall.rearrange("p (g k r) -> p g k r", g=G, k=KK)

    acc0 = consts.tile([128, C * RPC], DT)
    acc1 = consts.tile([128, C * RPC], DT)

    # MAC: loop dw outer (DMA'd slab pair), dh inner
    first = {0: True, 1: True}
    for dw in range(K):
        xe = slabs.tile([128, XF], DT, tag="xe")
        nc.sync.dma_start(out=xe, in_=xpad_d[dw : dw + 128, :])
        xo = slabs.tile([128, XF], DT, tag="xo")
        nc.sync.dma_start(out=xo, in_=xpod_d[dw : dw + 128, :])
        xe3 = xe.rearrange("p (c r) -> p c r", r=XR)
        xo3 = xo.rearrange("p (c r) -> p c r", r=XR)
        for dh in range(K):
            k = dh * K + dw
            if dh % 2 == 0:
                xop = xe3[:, :, dh : dh + RPC]
            else:
                xop = xo3[:, :, dh - 1 : dh - 1 + RPC]
            wop = wall4[:, :, k : k + 1, :].broadcast_to([128, G, GC, RPC])
            a = acc0 if k % 2 == 0 else acc1
            if first[k % 2]:
                nc.vector.tensor_mul(a, xop, wop)
                first[k % 2] = False
            else:
                t = tmppool.tile([128, C * RPC], DT, tag="t")
                nc.vector.tensor_mul(t, xop, wop)
                nc.vector.tensor_add(a, a, t)

    outf = consts.tile([128, C * RPC], DT)
    QF = C * RPC // 4
    for q in range(4):
        sl = slice(q * QF, (q + 1) * QF)
        nc.vector.tensor_add(outf[:, sl], acc0[:, sl], acc1[:, sl])
        nc.sync.dma_start(out=out_d[:, sl], in_=outf[:, sl])
    ctx.close()


def prep_inputs(x, conv_w, bn_gamma, bn_beta, bn_mean, bn_var):
    """Host-side prep: per-core padded pixel-major slabs + BN-folded weights."""
    bf = ml_dtypes.bfloat16
    scale = (bn_gamma / np.sqrt(bn_var + BN_EPS)).astype(np.float32)
    shift = (bn_beta - bn_mean * scale).astype(np.float32)

    # BN-folded transposed conv weights [65, 196]: rows 0..63 = (scale*W)^T,
    # row 64 = shift (pairs with the ones-row in xcm).
    wconv = np.zeros((C + 1, KO), np.float32)
    wconv[0:C] = (conv_w * scale[:, None]).T
    wconv[C] = shift
    wconv = wconv.astype(bf)

    # padded image [B, C, H+6, W+6]
    xp = np.zeros((B, C, H + 2 * PAD, W + 2 * PAD), np.float32)
    xp[:, :, PAD : PAD + H, PAD : PAD + W] = x

    in_maps = []
    for core in range(8):
        b, half = core // 2, core % 2
        h0 = 64 * half
        # xpad [134 wcols, (c, 70 rows)]: rows h0-3 .. h0+66 (padded idx h0..h0+69)
        slab = xp[b, :, h0 : h0 + XR, :]              # [C, 70, 134]
        xpad = np.ascontiguousarray(slab.transpose(2, 0, 1)).reshape(WCOL, XF)
        xpad = xpad.astype(bf)
        xpod = np.zeros_like(xpad)
        xpod[:, : XF - 1] = xpad[:, 1:]
        # xcm [65, (r, w)]: channels of the 64 output rows + ones row
        xcm = np.zeros((C + 1, RPC, W), np.float32)
        xcm[0:C] = x[b, :, h0 : h0 + RPC, :]
        xcm[C] = 1.0
        in_maps.append(
            {
                "xpad": xpad,
                "xpod": xpod,
                "xcm": xcm.reshape(C + 1, RPC * W).astype(bf),
                "wconv": wconv,
            }
        )
    return in_maps


def assemble_output(results):
    out = np.zeros((B, C, H, W), np.float32)
    for core in range(8):
        b, half = core // 2, core % 2
        h0 = 64 * half
        arr = results[core]["out"].astype(np.float32).reshape(128, C, RPC)  # [w, c, r]
        out[b, :, h0 : h0 + RPC, :] = arr.transpose(1, 2, 0)
    return out


def kernel(x, conv_w, bn_gamma, bn_beta, bn_mean, bn_var):
    x = np.asarray(x, np.float32)
    conv_w = np.asarray(conv_w, np.float32)
    in_maps = prep_inputs(
        x,
        conv_w,
        np.asarray(bn_gamma, np.float32),
        np.asarray(bn_beta, np.float32),
        np.asarray(bn_mean, np.float32),
        np.asarray(bn_var, np.float32),
    )
    nc = build_bass()
    res = run_bass_kernel_spmd(nc, in_maps, core_ids=list(range(8)))
    return assemble_output(res.results)


if __name__ == "__main__":
    rng = np.random.default_rng(0)
    ins = {
        "x": rng.standard_normal((B, C, H, W), np.float32),
        "conv_w": rng.standard_normal((KO, C), np.float32) / 8.0,
        "bn_gamma": rng.uniform(0.5, 1.5, KO).astype(np.float32),
        "bn_beta": rng.standard_normal(KO).astype(np.float32) * 0.1,
        "bn_mean": rng.standard_normal(KO).astype(np.float32) * 0.1,
        "bn_var": rng.uniform(0.5, 1.5, KO).astype(np.float32),
    }
    out = kernel(**ins)
    print("kernel output", out.shape, out.dtype, np.abs(out).sum())


# revision 28
# speedup vs baseline: 1.0228x; 1.0217x over previous
"""Trainium2 Bass kernel for the Involution module (B=4, C=64, H=W=128, K=7, G=4).

v2 pixel-major architecture (8-way data parallel: core = (batch, h-half)):
  - partitions = 128 w-columns; free dim = (channel, row).
  - 1x1 kernel-generating conv runs TRANSPOSED on TensorE: lhsT = x-slice
    [65, 128] (64 channels + ones row), rhs = [65, 196] BN-folded weights;
    out z[128 px, 196 ko] in PSUM. SiLU on ScalarE -> bf16 `wall` laid out
    [p, (ko, r)] with r innermost (strided activation writes).
  - involution MAC on DVE: per k=(dh,dw), tensor_mul with the w operand read
    via a stride-0 broadcast AP over the 16 group-channels (no replication
    matmuls, no PSUM drain). dh shifts = free-dim offsets (odd dh uses an
    element-shifted DMA variant to keep bf16 2x alignment); dw shifts =
    DMA'd row-offset slabs from a 134-row padded DRAM image.
"""

import numpy as np
import ml_dtypes

import concourse.bacc as bacc
import concourse.tile as tile
import concourse.mybir as mybir
from concourse.bass_utils import run_bass_kernel_spmd

# Problem constants (hardcoded per harness contract).
B, C, H, W = 4, 64, 128, 128
K, G, GC = 7, 4, 16
KK = K * K
KO = KK * G  # 196
PAD = 3
BN_EPS = 1e-5

RPC = 64          # output rows per core
XR = RPC + 2 * PAD  # 70 rows incl. dh halo
XF = C * XR       # 4480 free elems per x slab partition
WCOL = W + 2 * PAD  # 134 padded w-columns in DRAM


def build_bass():
    nc = bacc.Bacc(
        "TRN2",
        target_bir_lowering=False,
        debug=False,
        enable_asserts=False,
        num_devices=8,
    )
    DT = mybir.dt.bfloat16
    f32 = mybir.dt.float32

    xpad_d = nc.dram_tensor("xpad", [WCOL, XF], DT, kind="ExternalInput").ap()
    xpod_d = nc.dram_tensor("xpod", [WCOL, XF], DT, kind="ExternalInput").ap()
    xcm_d = nc.dram_tensor("xcm", [C + 1, RPC * W], DT, kind="ExternalInput").ap()
    wconv_d = nc.dram_tensor("wconv", [C + 1, KO], DT, kind="ExternalInput").ap()
    out_d = nc.dram_tensor("out", [128, C * RPC], DT, kind="ExternalOutput").ap()

    with tile.TileContext(nc) as tc:
        build_kernel(tc, xpad_d, xpod_d, xcm_d, wconv_d, out_d)
    nc.compile()
    return nc


def build_kernel(tc, xpad_d, xpod_d, xcm_d, wconv_d, out_d):
    from contextlib import ExitStack

    nc = tc.nc
    DT = mybir.dt.bfloat16
    f32 = mybir.dt.float32
    silu = mybir.ActivationFunctionType.Silu

    ctx = ExitStack()
    consts = ctx.enter_context(tc.tile_pool(name="consts", bufs=1))
    slabs = ctx.enter_context(tc.tile_pool(name="slabs", bufs=4))
    tmppool = ctx.enter_context(tc.tile_pool(name="tmp", bufs=3))
    zpool = ctx.enter_context(tc.tile_pool(name="z", bufs=8, space="PSUM"))

    wconv = consts.tile([C + 1, KO], DT)
    nc.sync.dma_start(out=wconv, in_=wconv_d)
    # xcm as 16-row quarter tiles: conv rows start as soon as their slice lands
    NQ = 4
    QR = RPC // NQ
    xcmq = []
    for q in range(NQ):
        xq = consts.tile([C + 1, QR * W], DT, name=f"xcmq{q}")
        nc.sync.dma_start(out=xq, in_=xcm_d[:, q * QR * W : (q + 1) * QR * W])
        xcmq.append(xq)

    # conv: 64 transposed matmuls + SiLU into zbuf[p, (r, ko)] (contiguous act
    # writes), then DVE relayout to wall[p, (ko, r)] (r innermost for the
    # 2x-mode MAC), chunked so the transpose overlaps the remaining acts.
    zbuf = consts.tile([128, RPC * KO], DT)
    zbuf3 = zbuf.rearrange("p (r ko) -> p r ko", r=RPC)
    wall = consts.tile([128, KO * RPC], DT)
    wall3 = wall.rearrange("p (ko r) -> p ko r", r=RPC)
    RCH = 8
    for r0 in range(0, RPC, RCH):
        for rb in range(r0, r0 + RCH, 2):
            zr = zpool.tile([128, 2 * KO], f32, tag="z")
            for i in range(2):
                r = rb + i
                nc.tensor.matmul(
                    zr[:, i * KO : (i + 1) * KO],
                    xcmq[r // QR][:, (r % QR) * W : (r % QR + 1) * W],
                    wconv,
                    start=True,
                    stop=True,
                )
            nc.scalar.activation(zbuf3[:, rb : rb + 2, :], zr, silu)
        # [RCH, KO] -> [KO, RCH] free-dim transpose (DVE 1x strided copy)
        nc.vector.tensor_copy(
            wall3[:, :, r0 : r0 + RCH],
            zbuf3[:, r0 : r0 + RCH, :].transpose([0, 2, 1]),
        )

    wall4 = wall.rearrange("p (g k r) -> p g k r", g=G, k=KK)

    acc0 = consts.tile([128, C * RPC], DT)
    acc1 = consts.tile([128, C * RPC], DT)

    # MAC: loop dw outer (DMA'd slab pair), dh inner
    first = {0: True, 1: True}
    for dw in range(K):
        xe = slabs.tile([128, XF], DT, tag="xe")
        nc.sync.dma_start(out=xe, in_=xpad_d[dw : dw + 128, :])
        xo = slabs.tile([128, XF], DT, tag="xo")
        nc.sync.dma_start(out=xo, in_=xpod_d[dw : dw + 128, :])
        xe3 = xe.rearrange("p (c r) -> p c r", r=XR)
        xo3 = xo.rearrange("p (c r) -> p c r", r=XR)
        for dh in range(K):
            k = dh * K + dw
            if dh % 2 == 0:
                xop = xe3[:, :, dh : dh + RPC]
            else:
                xop = xo3[:, :, dh - 1 : dh - 1 + RPC]
            wop = wall4[:, :, k : k + 1, :].broadcast_to([128, G, GC, RPC])
            a = acc0 if k % 2 == 0 else acc1
            if first[k % 2]:
                nc.vector.tensor_mul(a, xop, wop)
                first[k % 2] = False
            else:
                t = tmppool.tile([128, C * RPC], DT, tag="t")
                nc.vector.tensor_mul(t, xop, wop)
                nc.vector.tensor_add(a, a, t)

    outf = consts.tile([128, C * RPC], DT)
    QF = C * RPC // 4
    for q in range(4):
        sl = slice(q * QF, (q + 1) * QF)
        nc.vector.tensor_add(outf[:, sl], acc0[:, sl], acc1[:, sl])
        nc.sync.dma_start(out=out_d[:, sl], in_=outf[:, sl])
    ctx.close()


def prep_inputs(x, conv_w, bn_gamma, bn_beta, bn_mean, bn_var):
    """Host-side prep: per-core padded pixel-major slabs + BN-folded weights."""
    bf = ml_dtypes.bfloat16
    scale = (bn_gamma / np.sqrt(bn_var + BN_EPS)).astype(np.float32)
    shift = (bn_beta - bn_mean * scale).astype(np.float32)

    # BN-folded transposed conv weights [65, 196]: rows 0..63 = (scale*W)^T,
    # row 64 = shift (pairs with the ones-row in xcm).
    wconv = np.zeros((C + 1, KO), np.float32)
    wconv[0:C] = (conv_w * scale[:, None]).T
    wconv[C] = shift
    wconv = wconv.astype(bf)

    # padded image [B, C, H+6, W+6]
    xp = np.zeros((B, C, H + 2 * PAD, W + 2 * PAD), np.float32)
    xp[:, :, PAD : PAD + H, PAD : PAD + W] = x

    in_maps = []
    for core in range(8):
        b, half = core // 2, core % 2
        h0 = 64 * half
        # xpad [134 wcols, (c, 70 rows)]: rows h0-3 .. h0+66 (padded idx h0..h0+69)
        slab = xp[b, :, h0 : h0 + XR, :]              # [C, 70, 134]
        xpad = np.ascontiguousarray(slab.transpose(2, 0, 1)).reshape(WCOL, XF)
        xpad = xpad.astype(bf)
        xpod = np.zeros_like(xpad)
        xpod[:, : XF - 1] = xpad[:, 1:]
        # xcm [65, (r, w)]: channels of the 64 output rows + ones row
        xcm = np.zeros((C + 1, RPC, W), np.float32)
        xcm[0:C] = x[b, :, h0 : h0 + RPC, :]
        xcm[C] = 1.0
        in_maps.append(
            {
                "xpad": xpad,
                "xpod": xpod,
                "xcm": xcm.reshape(C + 1, RPC * W).astype(bf),
                "wconv": wconv,
            }
        )
    return in_maps


def assemble_output(results):
    out = np.zeros((B, C, H, W), np.float32)
    for core in range(8):
        b, half = core // 2, core % 2
        h0 = 64 * half
        arr = results[core]["out"].astype(np.float32).reshape(128, C, RPC)  # [w, c, r]
        out[b, :, h0 : h0 + RPC, :] = arr.transpose(1, 2, 0)
    return out


def kernel(x, conv_w, bn_gamma, bn_beta, bn_mean, bn_var):
    x = np.asarray(x, np.float32)
    conv_w = np.asarray(conv_w, np.float32)
    in_maps = prep_inputs(
        x,
        conv_w,
        np.asarray(bn_gamma, np.float32),
        np.asarray(bn_beta, np.float32),
        np.asarray(bn_mean, np.float32),
        np.asarray(bn_var, np.float32),
    )
    nc = build_bass()
    res = run_bass_kernel_spmd(nc, in_maps, core_ids=list(range(8)))
    return assemble_output(res.results)


if __name__ == "__main__":
    rng = np.random.default_rng(0)
    ins = {
        "x": rng.standard_normal((B, C, H, W), np.float32),
        "conv_w": rng.standard_normal((KO, C), np.float32) / 8.0,
        "bn_gamma": rng.uniform(0.5, 1.5, KO).astype(np.float32),
        "bn_beta": rng.standard_normal(KO).astype(np.float32) * 0.1,
        "bn_mean": rng.standard_normal(KO).astype(np.float32) * 0.1,
        "bn_var": rng.uniform(0.5, 1.5, KO).astype(np.float32),
    }
    out = kernel(**ins)
    print("kernel output", out.shape, out.dtype, np.abs(out).sum())


# revision 29
# speedup vs baseline: 1.0239x; 1.0011x over previous
"""Trainium2 Bass kernel for the Involution module (B=4, C=64, H=W=128, K=7, G=4).

v2 pixel-major architecture (8-way data parallel: core = (batch, h-half)):
  - partitions = 128 w-columns; free dim = (channel, row).
  - 1x1 kernel-generating conv runs TRANSPOSED on TensorE: lhsT = x-slice
    [65, 128] (64 channels + ones row), rhs = [65, 196] BN-folded weights;
    out z[128 px, 196 ko] in PSUM. SiLU on ScalarE -> bf16 `wall` laid out
    [p, (ko, r)] with r innermost (strided activation writes).
  - involution MAC on DVE: per k=(dh,dw), tensor_mul with the w operand read
    via a stride-0 broadcast AP over the 16 group-channels (no replication
    matmuls, no PSUM drain). dh shifts = free-dim offsets (odd dh uses an
    element-shifted DMA variant to keep bf16 2x alignment); dw shifts =
    DMA'd row-offset slabs from a 134-row padded DRAM image.
"""

import numpy as np
import ml_dtypes

import concourse.bacc as bacc
import concourse.tile as tile
import concourse.mybir as mybir
from concourse.bass_utils import run_bass_kernel_spmd

# Problem constants (hardcoded per harness contract).
B, C, H, W = 4, 64, 128, 128
K, G, GC = 7, 4, 16
KK = K * K
KO = KK * G  # 196
PAD = 3
BN_EPS = 1e-5

RPC = 64          # output rows per core
XR = RPC + 2 * PAD  # 70 rows incl. dh halo
XF = C * XR       # 4480 free elems per x slab partition
WCOL = W + 2 * PAD  # 134 padded w-columns in DRAM


def build_bass():
    nc = bacc.Bacc(
        "TRN2",
        target_bir_lowering=False,
        debug=False,
        enable_asserts=False,
        num_devices=8,
    )
    DT = mybir.dt.bfloat16
    f32 = mybir.dt.float32

    xpad_d = nc.dram_tensor("xpad", [WCOL, XF], DT, kind="ExternalInput").ap()
    xpod_d = nc.dram_tensor("xpod", [WCOL, XF], DT, kind="ExternalInput").ap()
    xcm_d = nc.dram_tensor("xcm", [C + 1, RPC * W], DT, kind="ExternalInput").ap()
    wconv_d = nc.dram_tensor("wconv", [C + 1, KO], DT, kind="ExternalInput").ap()
    out_d = nc.dram_tensor("out", [128, C * RPC], DT, kind="ExternalOutput").ap()

    with tile.TileContext(nc) as tc:
        build_kernel(tc, xpad_d, xpod_d, xcm_d, wconv_d, out_d)
    nc.compile()
    return nc


def build_kernel(tc, xpad_d, xpod_d, xcm_d, wconv_d, out_d):
    from contextlib import ExitStack

    nc = tc.nc
    DT = mybir.dt.bfloat16
    f32 = mybir.dt.float32
    silu = mybir.ActivationFunctionType.Silu

    ctx = ExitStack()
    consts = ctx.enter_context(tc.tile_pool(name="consts", bufs=1))
    slabs = ctx.enter_context(tc.tile_pool(name="slabs", bufs=4))
    tmppool = ctx.enter_context(tc.tile_pool(name="tmp", bufs=3))
    zpool = ctx.enter_context(tc.tile_pool(name="z", bufs=4, space="PSUM"))

    wconv = consts.tile([C + 1, KO], DT)
    nc.sync.dma_start(out=wconv, in_=wconv_d)
    # xcm as 16-row quarter tiles: conv rows start as soon as their slice lands
    NQ = 4
    QR = RPC // NQ
    xcmq = []
    for q in range(NQ):
        xq = consts.tile([C + 1, QR * W], DT, name=f"xcmq{q}")
        nc.sync.dma_start(out=xq, in_=xcm_d[:, q * QR * W : (q + 1) * QR * W])
        xcmq.append(xq)

    # conv: 64 transposed matmuls + SiLU into zbuf[p, (r, ko)] (contiguous act
    # writes), then DVE relayout to wall[p, (ko, r)] (r innermost for the
    # 2x-mode MAC), chunked so the transpose overlaps the remaining acts.
    zbuf = consts.tile([128, RPC * KO], DT)
    zbuf3 = zbuf.rearrange("p (r ko) -> p r ko", r=RPC)
    wall = consts.tile([128, KO * RPC], DT)
    wall3 = wall.rearrange("p (ko r) -> p ko r", r=RPC)
    RCH = 8
    for r0 in range(0, RPC, RCH):
        for rb in range(r0, r0 + RCH, 4):
            zr = zpool.tile([128, 4 * KO], f32, tag="z")
            for i in range(4):
                r = rb + i
                nc.tensor.matmul(
                    zr[:, i * KO : (i + 1) * KO],
                    xcmq[r // QR][:, (r % QR) * W : (r % QR + 1) * W],
                    wconv,
                    start=True,
                    stop=True,
                )
            nc.scalar.activation(zbuf3[:, rb : rb + 4, :], zr, silu)
        # [RCH, KO] -> [KO, RCH] free-dim transpose (DVE 1x strided copy)
        nc.vector.tensor_copy(
            wall3[:, :, r0 : r0 + RCH],
            zbuf3[:, r0 : r0 + RCH, :].transpose([0, 2, 1]),
        )

    wall4 = wall.rearrange("p (g k r) -> p g k r", g=G, k=KK)

    acc0 = consts.tile([128, C * RPC], DT)
    acc1 = consts.tile([128, C * RPC], DT)

    # MAC: loop dw outer (DMA'd slab pair), dh inner
    first = {0: True, 1: True}
    for dw in range(K):
        xe = slabs.tile([128, XF], DT, tag="xe")
        nc.sync.dma_start(out=xe, in_=xpad_d[dw : dw + 128, :])
        xo = slabs.tile([128, XF], DT, tag="xo")
        nc.sync.dma_start(out=xo, in_=xpod_d[dw : dw + 128, :])
        xe3 = xe.rearrange("p (c r) -> p c r", r=XR)
        xo3 = xo.rearrange("p (c r) -> p c r", r=XR)
        for dh in range(K):
            k = dh * K + dw
            if dh % 2 == 0:
                xop = xe3[:, :, dh : dh + RPC]
            else:
                xop = xo3[:, :, dh - 1 : dh - 1 + RPC]
            wop = wall4[:, :, k : k + 1, :].broadcast_to([128, G, GC, RPC])
            a = acc0 if k % 2 == 0 else acc1
            if first[k % 2]:
                nc.vector.tensor_mul(a, xop, wop)
                first[k % 2] = False
            else:
                t = tmppool.tile([128, C * RPC], DT, tag="t")
                nc.vector.tensor_mul(t, xop, wop)
                nc.vector.tensor_add(a, a, t)

    outf = consts.tile([128, C * RPC], DT)
    QF = C * RPC // 4
    for q in range(4):
        sl = slice(q * QF, (q + 1) * QF)
        nc.vector.tensor_add(outf[:, sl], acc0[:, sl], acc1[:, sl])
        nc.sync.dma_start(out=out_d[:, sl], in_=outf[:, sl])
    ctx.close()


def prep_inputs(x, conv_w, bn_gamma, bn_beta, bn_mean, bn_var):
    """Host-side prep: per-core padded pixel-major slabs + BN-folded weights."""
    bf = ml_dtypes.bfloat16
    scale = (bn_gamma / np.sqrt(bn_var + BN_EPS)).astype(np.float32)
    shift = (bn_beta - bn_mean * scale).astype(np.float32)

    # BN-folded transposed conv weights [65, 196]: rows 0..63 = (scale*W)^T,
    # row 64 = shift (pairs with the ones-row in xcm).
    wconv = np.zeros((C + 1, KO), np.float32)
    wconv[0:C] = (conv_w * scale[:, None]).T
    wconv[C] = shift
    wconv = wconv.astype(bf)

    # padded image [B, C, H+6, W+6]
    xp = np.zeros((B, C, H + 2 * PAD, W + 2 * PAD), np.float32)
    xp[:, :, PAD : PAD + H, PAD : PAD + W] = x

    in_maps = []
    for core in range(8):
        b, half = core // 2, core % 2
        h0 = 64 * half
        # xpad [134 wcols, (c, 70 rows)]: rows h0-3 .. h0+66 (padded idx h0..h0+69)
        slab = xp[b, :, h0 : h0 + XR, :]              # [C, 70, 134]
        xpad = np.ascontiguousarray(slab.transpose(2, 0, 1)).reshape(WCOL, XF)
        xpad = xpad.astype(bf)
        xpod = np.zeros_like(xpad)
        xpod[:, : XF - 1] = xpad[:, 1:]
        # xcm [65, (r, w)]: channels of the 64 output rows + ones row
        xcm = np.zeros((C + 1, RPC, W), np.float32)
        xcm[0:C] = x[b, :, h0 : h0 + RPC, :]
        xcm[C] = 1.0
        in_maps.append(
            {
                "xpad": xpad,
                "xpod": xpod,
                "xcm": xcm.reshape(C + 1, RPC * W).astype(bf),
                "wconv": wconv,
            }
        )
    return in_maps


def assemble_output(results):
    out = np.zeros((B, C, H, W), np.float32)
    for core in range(8):
        b, half = core // 2, core % 2
        h0 = 64 * half
        arr = results[core]["out"].astype(np.float32).reshape(128, C, RPC)  # [w, c, r]
        out[b, :, h0 : h0 + RPC, :] = arr.transpose(1, 2, 0)
    return out


def kernel(x, conv_w, bn_gamma, bn_beta, bn_mean, bn_var):
    x = np.asarray(x, np.float32)
    conv_w = np.asarray(conv_w, np.float32)
    in_maps = prep_inputs(
        x,
        conv_w,
        np.asarray(bn_gamma, np.float32),
        np.asarray(bn_beta, np.float32),
        np.asarray(bn_mean, np.float32),
        np.asarray(bn_var, np.float32),
    )
    nc = build_bass()
    res = run_bass_kernel_spmd(nc, in_maps, core_ids=list(range(8)))
    return assemble_output(res.results)


if __name__ == "__main__":
    rng = np.random.default_rng(0)
    ins = {
        "x": rng.standard_normal((B, C, H, W), np.float32),
        "conv_w": rng.standard_normal((KO, C), np.float32) / 8.0,
        "bn_gamma": rng.uniform(0.5, 1.5, KO).astype(np.float32),
        "bn_beta": rng.standard_normal(KO).astype(np.float32) * 0.1,
        "bn_mean": rng.standard_normal(KO).astype(np.float32) * 0.1,
        "bn_var": rng.uniform(0.5, 1.5, KO).astype(np.float32),
    }
    out = kernel(**ins)
    print("kernel output", out.shape, out.dtype, np.abs(out).sum())


# revision 32
# speedup vs baseline: 1.0286x; 1.0047x over previous
"""Trainium2 Bass kernel for the Involution module (B=4, C=64, H=W=128, K=7, G=4).

v2 pixel-major architecture (8-way data parallel: core = (batch, h-half)):
  - partitions = 128 w-columns; free dim = (channel, row).
  - 1x1 kernel-generating conv runs TRANSPOSED on TensorE: lhsT = x-slice
    [65, 128] (64 channels + ones row), rhs = [65, 196] BN-folded weights;
    out z[128 px, 196 ko] in PSUM. SiLU on ScalarE -> bf16 `wall` laid out
    [p, (ko, r)] with r innermost (strided activation writes).
  - involution MAC on DVE: per k=(dh,dw), tensor_mul with the w operand read
    via a stride-0 broadcast AP over the 16 group-channels (no replication
    matmuls, no PSUM drain). dh shifts = free-dim offsets (odd dh uses an
    element-shifted DMA variant to keep bf16 2x alignment); dw shifts =
    DMA'd row-offset slabs from a 134-row padded DRAM image.
"""

import os

os.environ.setdefault("NEURON_RT_RESET_CORES", "1")

import numpy as np
import ml_dtypes

import concourse.bacc as bacc
import concourse.tile as tile
import concourse.mybir as mybir
from concourse.bass_utils import run_bass_kernel_spmd

# Problem constants (hardcoded per harness contract).
B, C, H, W = 4, 64, 128, 128
K, G, GC = 7, 4, 16
KK = K * K
KO = KK * G  # 196
PAD = 3
BN_EPS = 1e-5

RPC = 64          # output rows per core
XR = RPC + 2 * PAD  # 70 rows incl. dh halo
XF = C * XR       # 4480 free elems per x slab partition
WCOL = W + 2 * PAD  # 134 padded w-columns in DRAM


def build_bass():
    nc = bacc.Bacc(
        "TRN2",
        target_bir_lowering=False,
        debug=False,
        enable_asserts=False,
        num_devices=8,
    )
    DT = mybir.dt.bfloat16
    f32 = mybir.dt.float32

    xpad_d = nc.dram_tensor("xpad", [WCOL, XF], DT, kind="ExternalInput").ap()
    xpod_d = nc.dram_tensor("xpod", [WCOL, XF], DT, kind="ExternalInput").ap()
    xcm_d = nc.dram_tensor("xcm", [C + 1, RPC * W], DT, kind="ExternalInput").ap()
    wconv_d = nc.dram_tensor("wconv", [C + 1, KO], DT, kind="ExternalInput").ap()
    out_d = nc.dram_tensor("out", [128, C * RPC], DT, kind="ExternalOutput").ap()

    with tile.TileContext(nc) as tc:
        build_kernel(tc, xpad_d, xpod_d, xcm_d, wconv_d, out_d)
    nc.compile()
    return nc


def build_kernel(tc, xpad_d, xpod_d, xcm_d, wconv_d, out_d):
    from contextlib import ExitStack

    nc = tc.nc
    DT = mybir.dt.bfloat16
    f32 = mybir.dt.float32
    silu = mybir.ActivationFunctionType.Silu

    ctx = ExitStack()
    consts = ctx.enter_context(tc.tile_pool(name="consts", bufs=1))
    slabs = ctx.enter_context(tc.tile_pool(name="slabs", bufs=4))
    tmppool = ctx.enter_context(tc.tile_pool(name="tmp", bufs=3))
    zpool = ctx.enter_context(tc.tile_pool(name="z", bufs=4, space="PSUM"))

    wconv = consts.tile([C + 1, KO], DT)
    nc.sync.dma_start(out=wconv, in_=wconv_d)
    # xcm as 16-row quarter tiles: conv rows start as soon as their slice lands
    NQ = 4
    QR = RPC // NQ
    xcmq = []
    for q in range(NQ):
        xq = consts.tile([C + 1, QR * W], DT, name=f"xcmq{q}")
        nc.sync.dma_start(out=xq, in_=xcm_d[:, q * QR * W : (q + 1) * QR * W])
        xcmq.append(xq)

    # conv: 64 transposed matmuls + SiLU into zbuf[p, (r, ko)] (contiguous act
    # writes), then DVE relayout to wall[p, (ko, r)] (r innermost for the
    # 2x-mode MAC), chunked so the transpose overlaps the remaining acts.
    zbuf = consts.tile([128, RPC * KO], DT)
    zbuf3 = zbuf.rearrange("p (r ko) -> p r ko", r=RPC)
    wall = consts.tile([128, KO * RPC], DT)
    wall3 = wall.rearrange("p (ko r) -> p ko r", r=RPC)
    RCH = 8
    for r0 in range(0, RPC, RCH):
        for rb in range(r0, r0 + RCH, 4):
            zr = zpool.tile([128, 4 * KO], f32, tag="z")
            for i in range(4):
                r = rb + i
                nc.tensor.matmul(
                    zr[:, i * KO : (i + 1) * KO],
                    xcmq[r // QR][:, (r % QR) * W : (r % QR + 1) * W],
                    wconv,
                    start=True,
                    stop=True,
                )
            nc.scalar.activation(zbuf3[:, rb : rb + 4, :], zr, silu)
        # [RCH, KO] -> [KO, RCH] free-dim transpose (DVE 1x strided copy)
        nc.vector.tensor_copy(
            wall3[:, :, r0 : r0 + RCH],
            zbuf3[:, r0 : r0 + RCH, :].transpose([0, 2, 1]),
        )

    wall4 = wall.rearrange("p (g k r) -> p g k r", g=G, k=KK)

    acc0 = consts.tile([128, C * RPC], DT)
    acc1 = consts.tile([128, C * RPC], DT)

    # MAC: loop dw outer (DMA'd slab pair), dh inner
    first = {0: True, 1: True}
    for dw in range(K):
        xe = slabs.tile([128, XF], DT, tag="xe")
        nc.sync.dma_start(out=xe, in_=xpad_d[dw : dw + 128, :])
        xo = slabs.tile([128, XF], DT, tag="xo")
        nc.sync.dma_start(out=xo, in_=xpod_d[dw : dw + 128, :])
        xe3 = xe.rearrange("p (c r) -> p c r", r=XR)
        xo3 = xo.rearrange("p (c r) -> p c r", r=XR)
        for dh in range(K):
            k = dh * K + dw
            if dh % 2 == 0:
                xop = xe3[:, :, dh : dh + RPC]
            else:
                xop = xo3[:, :, dh - 1 : dh - 1 + RPC]
            wop = wall4[:, :, k : k + 1, :].broadcast_to([128, G, GC, RPC])
            a = acc0 if k % 2 == 0 else acc1
            if first[k % 2]:
                nc.vector.tensor_mul(a, xop, wop)
                first[k % 2] = False
            else:
                t = tmppool.tile([128, C * RPC], DT, tag="t")
                nc.vector.tensor_mul(t, xop, wop)
                nc.vector.tensor_add(a, a, t)

    outf = consts.tile([128, C * RPC], DT)
    QF = C * RPC // 4
    for q in range(4):
        sl = slice(q * QF, (q + 1) * QF)
        nc.vector.tensor_add(outf[:, sl], acc0[:, sl], acc1[:, sl])
        nc.sync.dma_start(out=out_d[:, sl], in_=outf[:, sl])
    ctx.close()


def prep_inputs(x, conv_w, bn_gamma, bn_beta, bn_mean, bn_var):
    """Host-side prep: per-core padded pixel-major slabs + BN-folded weights."""
    bf = ml_dtypes.bfloat16
    scale = (bn_gamma / np.sqrt(bn_var + BN_EPS)).astype(np.float32)
    shift = (bn_beta - bn_mean * scale).astype(np.float32)

    # BN-folded transposed conv weights [65, 196]: rows 0..63 = (scale*W)^T,
    # row 64 = shift (pairs with the ones-row in xcm).
    wconv = np.zeros((C + 1, KO), np.float32)
    wconv[0:C] = (conv_w * scale[:, None]).T
    wconv[C] = shift
    wconv = wconv.astype(bf)

    # padded image [B, C, H+6, W+6]
    xp = np.zeros((B, C, H + 2 * PAD, W + 2 * PAD), np.float32)
    xp[:, :, PAD : PAD + H, PAD : PAD + W] = x

    in_maps = []
    for core in range(8):
        b, half = core // 2, core % 2
        h0 = 64 * half
        # xpad [134 wcols, (c, 70 rows)]: rows h0-3 .. h0+66 (padded idx h0..h0+69)
        slab = xp[b, :, h0 : h0 + XR, :]              # [C, 70, 134]
        xpad = np.ascontiguousarray(slab.transpose(2, 0, 1)).reshape(WCOL, XF)
        xpad = xpad.astype(bf)
        xpod = np.zeros_like(xpad)
        xpod[:, : XF - 1] = xpad[:, 1:]
        # xcm [65, (r, w)]: channels of the 64 output rows + ones row
        xcm = np.zeros((C + 1, RPC, W), np.float32)
        xcm[0:C] = x[b, :, h0 : h0 + RPC, :]
        xcm[C] = 1.0
        in_maps.append(
            {
                "xpad": xpad,
                "xpod": xpod,
                "xcm": xcm.reshape(C + 1, RPC * W).astype(bf),
                "wconv": wconv,
            }
        )
    return in_maps


def assemble_output(results):
    out = np.zeros((B, C, H, W), np.float32)
    for core in range(8):
        b, half = core // 2, core % 2
        h0 = 64 * half
        arr = results[core]["out"].astype(np.float32).reshape(128, C, RPC)  # [w, c, r]
        out[b, :, h0 : h0 + RPC, :] = arr.transpose(1, 2, 0)
    return out


def kernel(x, conv_w, bn_gamma, bn_beta, bn_mean, bn_var):
    x = np.asarray(x, np.float32)
    conv_w = np.asarray(conv_w, np.float32)
    in_maps = prep_inputs(
        x,
        conv_w,
        np.asarray(bn_gamma, np.float32),
        np.asarray(bn_beta, np.float32),
        np.asarray(bn_mean, np.float32),
        np.asarray(bn_var, np.float32),
    )
    nc = build_bass()
    res = run_bass_kernel_spmd(nc, in_maps, core_ids=list(range(8)))
    return assemble_output(res.results)


if __name__ == "__main__":
    rng = np.random.default_rng(0)
    ins = {
        "x": rng.standard_normal((B, C, H, W), np.float32),
        "conv_w": rng.standard_normal((KO, C), np.float32) / 8.0,
        "bn_gamma": rng.uniform(0.5, 1.5, KO).astype(np.float32),
        "bn_beta": rng.standard_normal(KO).astype(np.float32) * 0.1,
        "bn_mean": rng.standard_normal(KO).astype(np.float32) * 0.1,
        "bn_var": rng.uniform(0.5, 1.5, KO).astype(np.float32),
    }
    out = kernel(**ins)
    print("kernel output", out.shape, out.dtype, np.abs(out).sum())
